# revision 1
# baseline (speedup 1.0000x reference)
"""Trainium2 kernel for nn_BranchModel_9680856285960 (moe_routing).

Math: the reference scatters per-branch sparse weights into dense
(n_br, n_out, n_in) tensors, einsums against x, then takes a context-
gated masked sum over branches followed by relu.  Because the mask-
weighted branch sum commutes with the contraction over input features,
the whole model collapses to a 3-layer dense MLP

    out = relu(relu(x @ Weff1.T) @ Weff2.T) @ W3 + b3

where  Weff_l[o, i] = sum_{r,k} masks_l[ctx, r, o] * w_l[r, o, k]
                                * [idx_l[r, o, k] == i].

Reductions vs a naive port:
 - The effective-weight fold (scatter-add over 5.6M index/value pairs)
   is host-side, once per distinct input set.
 - Dead-unit pruning: with sparsity 0.8, ~11% of hidden units have all
   ten branch gates masked, so their Weff columns are exactly zero and
   the matching rows of the next layer contribute nothing.  Both hidden
   layers shrink 2000 -> ~1792 (padded to 128), cutting the dominant
   Weff2 HBM stream by ~20%.
 - Feature-major dataflow: activations live as [features, batch] so
   every layer contracts against resident feature-major tiles with the
   weight tile as the (stationary) lhsT -- no PE transposes anywhere.
   The PE's 64-deep reorder window hides the per-matmul LDWEIGHTS.
 - Weights are packed host-side into large contiguous DRAM blocks in
   output-chunk-major order and loaded with ~0.5-1MB chunked DMAs (few
   dma_starts: the per-op issue cost on the two HWDGE rings otherwise
   rate-limits the stream).  Each 512-column output chunk of a layer
   finishes (relu + next-layer partials) while later chunks stream.
 - A warm-up matmul spin on an uninitialized tile (no data deps, so it
   issues right after the engine preamble) lifts the PE HAM clock gate
   (1.2 -> 2.4 GHz) before the first real matmul.

Sharding: data-parallel over batch (8 cores x 128 rows), effective
weights replicated per core, fp16 stream with fp32 PSUM accumulation.
No collectives.  The kernel is HBM-bound on the ~9.7MB weight stream.
"""

import os
import sys
import numpy as np

for _p in ("/opt/trn_rl_repo",):
    if os.path.isdir(_p) and _p not in sys.path:
        sys.path.append(_p)

from contextlib import ExitStack

from concourse import bass, mybir
import concourse.bacc as bacc
import concourse.tile as tile
from concourse.bass_utils import run_bass_kernel_spmd

F32 = mybir.dt.float32
F16 = mybir.dt.float16

BATCH, NIN, NH_FULL, NOUT = 1024, 784, 2000, 10
NCORES = 8
BS = BATCH // NCORES            # 128 batch rows per core
P = 128


def _tiles(total, step):
    out, o = [], 0
    while o < total:
        out.append((o, min(step, total - o)))
        o += step
    return out


MT1 = _tiles(NIN, P)            # layer-1 contraction tiles: 6x128 + 16

LAST_RESULT = None
_CACHE = {}


def _build_weff(w, idx, mask_row, n_in):
    """Weff[o, i] = sum_{r,k} mask_row[r,o] * w[r,o,k] * [idx[r,o,k] == i]"""
    n_br, n_out, npb = w.shape
    acc = np.zeros(n_out * n_in, np.float64)
    base = (np.arange(n_out, dtype=np.int64) * n_in)[:, None]
    for r in range(n_br):
        flat = (base + idx[r].astype(np.int64)).ravel()
        vals = (w[r].astype(np.float64) * mask_row[r].astype(np.float64)[:, None]).ravel()
        acc += np.bincount(flat, weights=vals, minlength=n_out * n_in)
    return acc.reshape(n_out, n_in).astype(np.float32)


def _chunk_cols(ctiles, nchk):
    """Column base offsets of the output-chunk-major packed layout."""
    nbase, b = [], 0
    for _, nsz in nchk:
        nbase.append(b)
        b += len(ctiles) * nsz
    return nbase, b


def _mlp_body(tc, nh, xT, w1p, w2p, w3p, b3r, out):
    nc = tc.nc
    rings = [nc.sync, nc.scalar]          # the two HWDGE rings
    mt2 = _tiles(nh, P)                   # feature tiles of both hidden layers
    nchk = _tiles(nh, 512)                # output chunks (stream granularity)
    nt1, nt2 = len(MT1), len(mt2)
    n1base, cw1 = _chunk_cols(MT1, nchk)
    n2base, cw2 = _chunk_cols(mt2, nchk)

    with ExitStack() as ctx:
        const = ctx.enter_context(tc.tile_pool(name="const", bufs=1))
        act = ctx.enter_context(tc.tile_pool(name="act", bufs=1))
        pacc = ctx.enter_context(tc.tile_pool(name="pacc", bufs=1, space="PSUM"))

        # PE warm-up against the HAM clock gate: garbage-in, discarded-out.
        wz = const.tile([P, 512], F16, tag="warmz")
        nc.vector.memset(wz[:, :1], 0.0)   # minimal write so Tile allocates it
        pwarm = pacc.tile([P, 512], F32, tag="pwarm")
        # ~8 cold spins trip the HAM flip (~3.4us), the rest run warm and
        # deliberately delay compute until the full w1 slab has streamed --
        # the kernel is stream-bound, so only gapless PE execution matters.
        for _ in range(20):
            nc.tensor.matmul(pwarm[:], lhsT=wz[:, :P], rhs=wz[:],
                             start=True, stop=True)

        # x host-packed feature-major [128, nt1, BS]; first on the sync ring.
        xbig = const.tile([P, nt1, BS], F16, tag="xbig")
        nc.sync.dma_start(out=xbig[:], in_=xT)
        xts = [xbig[:sz, t, :] for t, (off, sz) in enumerate(MT1)]

        b3t = const.tile([NOUT, 1], F32, tag="b3")
        nc.gpsimd.dma_start(out=b3t[:], in_=b3r)
        w3t = const.tile([P, nt2, NOUT], F16, tag="w3")
        nc.gpsimd.dma_start(out=w3t[:], in_=w3p)

        # w1 slab-major [P, nt1, nh], one ~0.45MB DMA per slab alternating
        # rings (large single DMAs complete pathologically late; slab
        # granularity tracks consumption).  Tail slab partition-restricted.
        w1big = const.tile([P, nt1, nh], F16, tag="w1big")
        for t, (toff, tsz) in enumerate(MT1):
            rings[t % 2].dma_start(out=w1big[:tsz, t, :],
                                   in_=w1p[:tsz, t, :])

        # w2 streams output-chunk-major in ~0.45MB pieces alternating rings.
        w2big = const.tile([P, cw2], F16, tag="w2big")
        ndma = 0
        for n, (noff, nsz) in enumerate(nchk):
            b0 = n2base[n]
            b1 = b0 + nt2 * nsz
            step = max(1, (b1 - b0) // 4)
            c = b0
            while c < b1:
                hi = min(c + step, b1)
                rings[ndma % 2].dma_start(out=w2big[:, c:hi],
                                          in_=w2p[:, c:hi])
                ndma += 1
                c = hi

        def wview2(n, t, tsz, nsz, jl):
            c0 = n2base[n] + t * nsz + jl * P
            return w2big[:tsz, c0:c0 + P]

        def heartbeat(k):
            # dummy N=512 matmuls (213ns warm) that keep the PE HAM activity
            # window busy while the weight stream is the bottleneck.
            for _ in range(k):
                nc.tensor.matmul(pwarm[:], lhsT=wz[:, :P], rhs=wz[:],
                                 start=True, stop=True)


        # ---- Layer 1: h1 = relu(Weff1.T @ x), feature-major.
        # Wave 0 (first 4 chains) is paced by slab arrival; N=512 heartbeat
        # filler keeps the PE HAM activity window busy between arrivals so
        # the clock never drops to 1.2GHz.  Later waves run dense.
        h1big = act.tile([P, nt2, BS], F16, tag="h1big")
        psb = [pacc.tile([P, BS], F32, name=f"pp{i}", tag=f"pp{i}")
               for i in range(4)]
        nwave = (nt2 + 3) // 4
        for w in range(nwave):
            chains = [j for j in range(4 * w, min(4 * w + 4, nt2))]
            for t, (toff, tsz) in enumerate(MT1):
                for j in chains:
                    nc.tensor.matmul(
                        psb[j % 4][:],
                        lhsT=w1big[:tsz, t, j * P:(j + 1) * P],
                        rhs=xts[t],
                        start=(t == 0),
                        stop=(t == nt1 - 1),
                    )
                if w == 0:
                    heartbeat(5)
                elif w == 1:
                    heartbeat(1)
            for j in chains:
                if j % 2 == 0:
                    nc.vector.tensor_scalar_max(h1big[:, j, :],
                                                psb[j % 4][:], 0.0)
                else:
                    nc.scalar.activation(h1big[:, j, :], psb[j % 4][:],
                                         mybir.ActivationFunctionType.Relu)

        # ---- Layer 2 + fused layer 3: j-outer over output-chunk-major w2
        h2s = [act.tile([P, BS], F16, name=f"h2_{i}", tag=f"h2_{i}")
               for i in range(4)]
        ps3 = pacc.tile([NOUT, BS], F32, tag="ps3")
        ji = 0
        for n, (noff, nsz) in enumerate(nchk):
            for jl in range(nsz // P):
                ps = psb[ji % 4]
                for t, (toff, tsz) in enumerate(mt2):
                    nc.tensor.matmul(
                        ps[:],
                        lhsT=wview2(n, t, tsz, nsz, jl),
                        rhs=h1big[:tsz, t, :],
                        start=(t == 0),
                        stop=(t == nt2 - 1),
                    )
                h2 = h2s[ji % 4]
                if ji % 2 == 0:
                    nc.vector.tensor_scalar_max(h2[:], ps[:], 0.0)
                else:
                    nc.scalar.activation(h2[:], ps[:],
                                         mybir.ActivationFunctionType.Relu)
                nc.tensor.matmul(
                    ps3[:],
                    lhsT=w3t[:, ji, :],
                    rhs=h2[:],
                    start=(ji == 0),
                    stop=(ji == nt2 - 1),
                )
                ji += 1
                if n < len(nchk) - 1:
                    heartbeat(2)

        o = act.tile([NOUT, BS], F32, tag="o")
        nc.vector.tensor_add(o[:], ps3[:], b3t[:].to_broadcast([NOUT, BS]))
        nc.sync.dma_start(out=out, in_=o[:])


def _get_program(nh):
    key = ("nc", nh)
    if key in _CACHE:
        return _CACHE[key]
    nc = bacc.Bacc("TRN2", target_bir_lowering=False, debug=False,
                   enable_asserts=False, enable_partition_id=False,
                   num_devices=NCORES)
    mt2 = _tiles(nh, P)
    nchk = _tiles(nh, 512)
    _, cw1 = _chunk_cols(MT1, nchk)
    _, cw2 = _chunk_cols(mt2, nchk)
    xT = nc.dram_tensor("xT", [P, len(MT1), BS], F16,
                        kind="ExternalInput").ap()
    w1p = nc.dram_tensor("w1p", [P, len(MT1), nh], F16,
                         kind="ExternalInput").ap()
    w2p = nc.dram_tensor("w2p", [P, cw2], F16, kind="ExternalInput").ap()
    w3p = nc.dram_tensor("w3p", [P, len(mt2), NOUT], F16,
                         kind="ExternalInput").ap()
    b3r = nc.dram_tensor("b3r", [NOUT, 1], F32, kind="ExternalInput").ap()
    out = nc.dram_tensor("out", [NOUT, BS], F32, kind="ExternalOutput").ap()
    with tile.TileContext(nc) as tc:
        _mlp_body(tc, nh, xT, w1p, w2p, w3p, b3r, out)
    nc.compile()
    _CACHE[key] = nc
    return nc


def _pack_chunk_major(wt, ctiles, nchk):
    """wt [n_in, n_out] -> [128, cw] with block (chunk n, ctile t) holding
    wt[t*128+p, noff+j] at column base(n) + t*nsz + j."""
    nt = len(ctiles)
    nbase, cw = _chunk_cols(ctiles, nchk)
    out = np.zeros((P, cw), np.float16)
    for n, (noff, nsz) in enumerate(nchk):
        for t, (toff, tsz) in enumerate(ctiles):
            blk = wt[toff:toff + tsz, noff:noff + nsz]
            out[:tsz, nbase[n] + t * nsz: nbase[n] + (t + 1) * nsz] = blk
    return out


def kernel(x, w1, idx1, w2, idx2, masks1, masks2, W3, b3, context):
    global LAST_RESULT
    x = np.ascontiguousarray(np.asarray(x, dtype=np.float32))
    ctxi = int(np.asarray(context))
    m1 = np.asarray(masks1)[ctxi]
    m2 = np.asarray(masks2)[ctxi]

    # Dead-unit pruning.
    alive1 = np.where(m1.any(axis=0))[0]
    alive2 = np.where(m2.any(axis=0))[0]
    nh = max(len(alive1), len(alive2))
    nh = max(P, -(-nh // P) * P)

    weff1 = _build_weff(np.asarray(w1), np.asarray(idx1), m1, NIN)
    weff2 = _build_weff(np.asarray(w2), np.asarray(idx2), m2, NH_FULL)

    w1t = np.zeros((NIN, nh), np.float16)          # [in, out]
    w1t[:, :len(alive1)] = weff1[alive1, :].T.astype(np.float16)
    w2t = np.zeros((nh, nh), np.float16)           # [in, out]
    w2t[:len(alive1), :len(alive2)] = \
        weff2[np.ix_(alive2, alive1)].T.astype(np.float16)
    w3f = np.zeros((nh, NOUT), np.float16)
    w3f[:len(alive2)] = np.asarray(W3)[alive2, :].astype(np.float16)

    mt2 = _tiles(nh, P)
    nchk = _tiles(nh, 512)
    w1p = np.zeros((P, len(MT1), nh), np.float16)   # slab-major
    for t, (toff, tsz) in enumerate(MT1):
        w1p[:tsz, t, :] = w1t[toff:toff + tsz, :]
    w2p = _pack_chunk_major(w2t, mt2, nchk)
    w3p = np.zeros((P, len(mt2), NOUT), np.float16)
    for t, (off, sz) in enumerate(mt2):
        w3p[:sz, t, :] = w3f[off:off + sz, :]
    b3r = np.ascontiguousarray(
        np.asarray(b3, dtype=np.float32).reshape(NOUT, 1))

    try:
        import antenv.axon_hooks  # noqa: F401
    except Exception:
        os.environ.setdefault("BASS_NEVER_TRACE", "1")

    nc = _get_program(nh)
    in_maps = []
    for c in range(NCORES):
        xs = x[c * BS:(c + 1) * BS].T.astype(np.float16)
        xT = np.zeros((P, len(MT1), BS), np.float16)
        for t, (off, sz) in enumerate(MT1):
            xT[:sz, t, :] = xs[off:off + sz, :]
        in_maps.append({"xT": xT, "w1p": w1p, "w2p": w2p, "w3p": w3p,
                        "b3r": b3r})

    LAST_RESULT = run_bass_kernel_spmd(nc, in_maps, list(range(NCORES)))
    return np.concatenate(
        [LAST_RESULT.results[c]["out"].T for c in range(NCORES)], axis=0)

